# revision 16
# baseline (speedup 1.0000x reference)
"""Trainium2 Bass kernel for the AdaptiveGaussKronrod VJP quadrature problem.

Math (reference, flattened over N = S*15 = 1920 quadrature nodes):
    phi = sin(t (x) freqs)                  [N, D]
    Z   = phi @ W + b                       [N, D]
    G   = (h*wk)_n * cos(t (x) afreqs) * (1 - tanh(Z)^2)
    out = phi^T @ G                         [D, D]

Sharding: output-column parallel over 8 cores (J = D/8 = 512 columns each).
Core i needs W[:, cols], b[cols], afreqs[cols], full freqs. No collectives:
each core's [D, 512] output block is independent; host concatenates.

Per-core pipeline (Tile framework, bf16 matmuls / fp32 accumulation):
  pass 1 (GEMM1): phi_T tiles ([d, n] layout) generated by ScalarE Sin
    activation in 640-wide n-blocks; Z accumulated in PSUM per n-row-tile;
    epilogue computes G tiles [n, 512] via Tanh / Sin(pi/2 - x) / DVE math.
  pass 2 (GEMM2): phi_N tiles ([n, d] layout) regenerated by ScalarE in
    1024-wide d-column blocks; out accumulated in PSUM; DMA to DRAM.
All constant broadcast/column tiles are pre-arranged on the host so device
DMAs are contiguous. ScalarE emission interleaves phi generation with the
per-block epilogues so the in-order engine never blocks the PE.
"""

import math

import numpy as np

D = 4096
S = 128
J = D // 8          # output columns per core
N = S * 15          # 1920 quadrature nodes
P = 128
KT = D // P         # 32 k-tiles over D
MT = N // P         # 15 m-tiles over N
OT = D // P         # 32 output row tiles

PT_BLK_M = 5                     # pass-1 n-blocks: 3 x 640 (5 m-tiles each)
PT_BLK_W = PT_BLK_M * P          # 640
PT_NBLK = MT // PT_BLK_M         # 3
PN_BLK_O = 4                     # pass-2 d-col blocks: 8 x 512 (4 o-tiles)
PN_BLK_W = PN_BLK_O * P          # 512
PN_NBLK = OT // PN_BLK_O         # 8

_NODES_NEG = np.array([-0.9914553711208126, -0.9491079123427585, -0.8648644233597691,
                       -0.7415311855993945, -0.5860872354676911, -0.4058451513773972,
                       -0.20778495500789848, 0.0])
_WK_HALF = np.array([0.022935322010529224, 0.06309209262997856, 0.10479001032225019,
                     0.14065325971552592, 0.1690047266392679, 0.19035057806478542,
                     0.20443294007529889, 0.20948214108472782])
GK_NODES = np.concatenate([-_NODES_NEG[:-1][::-1], _NODES_NEG])  # [15]
GK_WK = np.concatenate([_WK_HALF[:-1][::-1], _WK_HALF])          # [15]


def _host_constants():
    edges = np.linspace(0.0, 1.0, S + 1, dtype=np.float32)
    a_s, b_s = edges[:-1], edges[1:]
    h = (b_s - a_s) / 2.0
    c = (a_s + b_s) / 2.0
    t = (c[:, None] + h[:, None] * GK_NODES[None, :].astype(np.float32)).reshape(-1)
    hw = (h[:, None] * GK_WK[None, :].astype(np.float32)).reshape(-1)
    return t.astype(np.float32), hw.astype(np.float32)


def _patch_act_tables():
    """Force Sin AND Tanh to resolve to one table set (silu_and_others) so
    the act-table-load pass emits a single load instead of thrashing
    between trig_and_small and exp_and_others on every Sin<->Tanh switch."""
    import concourse.bacc as bacc_mod
    from concourse import mybir

    if getattr(bacc_mod, "_act_tables_pinned", False):
        return
    orig = bacc_mod.get_activation_tables
    Sin = mybir.ActivationFunctionType.Sin
    Tanh = mybir.ActivationFunctionType.Tanh

    def patched(arch):
        tabs = orig(arch)
        out = {}
        for name, funcs in tabs.items():
            if (Sin in funcs) and (Tanh in funcs):
                out[name] = funcs
            else:
                out[name] = funcs - {Sin, Tanh}
        return out

    bacc_mod.get_activation_tables = patched
    bacc_mod._act_tables_pinned = True


def build_bass():
    """Build and compile the per-core Bass graph (identical on all 8 cores)."""
    from contextlib import ExitStack

    import concourse.bass as bass
    import concourse.tile as tile
    from concourse import bacc, mybir

    _patch_act_tables()

    f32 = mybir.dt.float32
    bf16 = mybir.dt.bfloat16
    Sin = mybir.ActivationFunctionType.Sin
    Tanh = mybir.ActivationFunctionType.Tanh

    nc = bacc.Bacc("TRN2", target_bir_lowering=False, debug=False,
                   enable_asserts=False)

    w_ext = nc.dram_tensor("w", [D, J], f32, kind="ExternalInput")
    tbc_ext = nc.dram_tensor("tbc", [P, N], f32, kind="ExternalInput")
    fbc_ext = nc.dram_tensor("fbc", [P, D], bf16, kind="ExternalInput")
    fpc_ext = nc.dram_tensor("fpc", [P, KT], f32, kind="ExternalInput")
    tpc_ext = nc.dram_tensor("tpc", [P, MT], f32, kind="ExternalInput")
    tnpc_ext = nc.dram_tensor("tnpc", [P, MT], f32, kind="ExternalInput")
    hwpc_ext = nc.dram_tensor("hwpc", [P, MT], f32, kind="ExternalInput")
    afbc_ext = nc.dram_tensor("afbc", [P, J], f32, kind="ExternalInput")
    bbc_ext = nc.dram_tensor("bbc", [P, J], f32, kind="ExternalInput")
    out_ext = nc.dram_tensor("out", [D, J], f32, kind="ExternalOutput")

    with tile.TileContext(nc) as tc, ExitStack() as ctx:
        consts = ctx.enter_context(tc.tile_pool(name="consts", bufs=1))
        stage = ctx.enter_context(tc.tile_pool(name="stage", bufs=3))
        wsp = ctx.enter_context(tc.tile_pool(name="ws", bufs=KT))
        phip = ctx.enter_context(tc.tile_pool(name="phi", bufs=64))
        work = ctx.enter_context(tc.tile_pool(name="work", bufs=2))
        gp = ctx.enter_context(tc.tile_pool(name="g", bufs=MT))
        zps = ctx.enter_context(
            tc.tile_pool(name="zpsum", bufs=6, space=bass.MemorySpace.PSUM))
        ops = ctx.enter_context(
            tc.tile_pool(name="opsum", bufs=2, space=bass.MemorySpace.PSUM))

        # ---- PE warm-up: dummy matmuls so HAM reaches K=8/8 before the
        # real GEMM starts (~3.4us of sustained PE activity required) ----
        dummy = consts.tile([P, J], bf16, tag="dummy")
        nc.vector.memset(dummy[:], 0.0)
        wps = ops.tile([P, J], f32, tag="opsum", name="warmps")
        for i in range(64):
            nc.tensor.matmul(wps[:, 0:64], lhsT=dummy[:, 0:128],
                             rhs=dummy[:, 128:192], start=True, stop=True)

        # ---- constants (host-prearranged, contiguous DMAs) ----
        t_bc = consts.tile([P, N], f32, tag="t_bc")
        nc.sync.dma_start(t_bc[:], tbc_ext[:])
        f_pc = consts.tile([P, KT], f32, tag="f_pc")
        nc.sync.dma_start(f_pc[:], fpc_ext[:])
        zero_c = consts.tile([P, 1], f32, tag="zero_c")
        nc.vector.memset(zero_c[:], 0.0)
        halfpi_c = consts.tile([P, 1], f32, tag="halfpi_c")
        nc.vector.memset(halfpi_c[:], math.pi / 2)
        # first ScalarE op: pulls the ACT table load to kernel start
        nc.scalar.activation(halfpi_c[:], zero_c[:], Sin, bias=zero_c[:])
        nc.vector.memset(halfpi_c[:], math.pi / 2)

        # ---- W shard: DMA f32, convert to bf16 (DVE) ----
        ws = []
        for k in range(KT):
            stg = stage.tile([P, J], f32, tag="stage512", name=f"wstg{k}")
            nc.sync.dma_start(stg[:], w_ext[k * P:(k + 1) * P, :])
            wb = wsp.tile([P, J], bf16, tag="ws", name=f"ws{k}")
            nc.vector.tensor_copy(wb[:], stg[:])
            ws.append(wb)

        # remaining constants (needed from the first epilogue onward)
        af_bc = consts.tile([P, J], f32, tag="af_bc")
        nc.sync.dma_start(af_bc[:], afbc_ext[:])
        b_bc = consts.tile([P, J], f32, tag="b_bc")
        nc.sync.dma_start(b_bc[:], bbc_ext[:])
        t_pc = consts.tile([P, MT], f32, tag="t_pc")
        nc.sync.dma_start(t_pc[:], tpc_ext[:])
        tn_pc = consts.tile([P, MT], f32, tag="tn_pc")
        nc.sync.dma_start(tn_pc[:], tnpc_ext[:])
        hw_pc = consts.tile([P, MT], f32, tag="hw_pc")
        nc.sync.dma_start(hw_pc[:], hwpc_ext[:])
        freqs_bc = consts.tile([P, D], bf16, tag="freqs_bc")
        nc.sync.dma_start(freqs_bc[:], fbc_ext[:])

        # ---- builders ----
        def gen_phit_block(blk):
            n0 = blk * PT_BLK_W
            tiles = []
            for k in range(KT):
                pt = phip.tile([P, PT_BLK_W], bf16, tag="phi",
                               name=f"pt{blk}_{k}")
                nc.scalar.activation(pt[:], t_bc[:, n0:n0 + PT_BLK_W], Sin,
                                     bias=zero_c[:], scale=f_pc[:, k:k + 1])
                tiles.append(pt)
            return tiles

        def gen_phin_block(blk):
            c0 = blk * PN_BLK_W
            tiles = []
            for n in range(MT):
                pn = phip.tile([P, PT_BLK_W], bf16, tag="phi",
                               name=f"pn{blk}_{n}")
                nc.scalar.activation(pn[:, :PN_BLK_W],
                                     freqs_bc[:, c0:c0 + PN_BLK_W], Sin,
                                     bias=zero_c[:], scale=t_pc[:, n:n + 1])
                tiles.append(pn)
            return tiles

        def mm_block(blk, phiT):
            zt = [zps.tile([P, J], f32, tag="zpsum", name=f"zt{blk}_{i}")
                  for i in range(PT_BLK_M)]
            for k in range(KT):
                for ml in range(PT_BLK_M):
                    nc.tensor.matmul(zt[ml][:],
                                     lhsT=phiT[k][:, ml * P:(ml + 1) * P],
                                     rhs=ws[k][:],
                                     start=(k == 0), stop=(k == KT - 1))
            return zt

        def epilogue(blk, zt):
            # z-adds first: frees all PSUM banks for the next block ASAP
            zs = []
            for ml in range(PT_BLK_M):
                z = work.tile([P, J], f32, tag="z", name=f"z{blk}_{ml}")
                nc.vector.tensor_add(z[:], zt[ml][:], b_bc[:])
                zs.append(z)
            for ml in range(PT_BLK_M):
                m = blk * PT_BLK_M + ml
                z = zs[ml]
                nc.scalar.activation(z[:], z[:], Tanh, bias=zero_c[:])
                c = work.tile([P, J], f32, tag="c", name=f"c{blk}_{ml}")
                nc.scalar.activation(c[:], af_bc[:], Sin,
                                     scale=tn_pc[:, m:m + 1], bias=halfpi_c[:])
                s = work.tile([P, J], f32, tag="s", name=f"s{blk}_{ml}")
                nc.vector.tensor_mul(s[:], z[:], z[:])
                nc.vector.tensor_scalar(s[:], s[:], -1.0, 1.0,
                                        mybir.AluOpType.mult, mybir.AluOpType.add)
                nc.vector.tensor_mul(c[:], c[:], s[:])
                g = gp.tile([P, J], bf16, tag="g", name=f"g{m}")
                nc.vector.tensor_scalar_mul(g[:], c[:], hw_pc[:, m:m + 1])
                g_tiles[m] = g

        g_tiles = [None] * MT
        # emission order chosen so the in-order ScalarE stream is:
        #   g0 g1 | e0 g2 | e1 p2g0 | e2 p2g1 | p2g2 p2g3
        phiT0 = gen_phit_block(0)
        phiT1 = gen_phit_block(1)
        zt0 = mm_block(0, phiT0)
        epilogue(0, zt0)
        phiT2 = gen_phit_block(2)
        zt1 = mm_block(1, phiT1)
        epilogue(1, zt1)
        phiN = {0: gen_phin_block(0)}
        zt2 = mm_block(2, phiT2)
        phiN[1] = gen_phin_block(1)
        epilogue(2, zt2)

        # ---- pass 2: GEMM2 (out = phi^T @ G) ----
        for blk in range(PN_NBLK):
            pn = phiN.pop(blk)
            if blk + 2 < PN_NBLK:
                phiN[blk + 2] = gen_phin_block(blk + 2)
            for ol in range(PN_BLK_O):
                o = blk * PN_BLK_O + ol
                op = ops.tile([P, J], f32, tag="opsum", name=f"op{o}")
                for n in range(MT):
                    nc.tensor.matmul(op[:],
                                     lhsT=pn[n][:, ol * P:(ol + 1) * P],
                                     rhs=g_tiles[n][:],
                                     start=(n == 0), stop=(n == MT - 1))
                ostg = stage.tile([P, J], f32, tag="stage512", name=f"ostg{o}")
                nc.vector.tensor_copy(ostg[:], op[:])
                nc.sync.dma_start(out_ext[o * P:(o + 1) * P, :], ostg[:])

    nc.compile()
    return nc


_CACHE = {}


def _get_nc():
    if "nc" not in _CACHE:
        _CACHE["nc"] = build_bass()
    return _CACHE["nc"]


def kernel(W, b, freqs, afreqs):
    import ml_dtypes
    from concourse.bass_utils import run_bass_kernel_spmd

    W = np.ascontiguousarray(np.asarray(W, dtype=np.float32))
    b = np.asarray(b, dtype=np.float32)
    freqs = np.asarray(freqs, dtype=np.float32)
    afreqs = np.asarray(afreqs, dtype=np.float32)
    t, hw = _host_constants()

    tbc = np.ascontiguousarray(np.broadcast_to(t[None, :], (P, N))).astype(np.float32)
    tpc = np.ascontiguousarray(t.reshape(MT, P).T)
    tnpc = np.ascontiguousarray((-t).reshape(MT, P).T)
    hwpc = np.ascontiguousarray(hw.reshape(MT, P).T)
    fbc = np.ascontiguousarray(
        np.broadcast_to(freqs[None, :], (P, D))).astype(ml_dtypes.bfloat16)
    fpc = np.ascontiguousarray(freqs.reshape(KT, P).T)

    nc = _get_nc()
    in_maps = []
    for i in range(8):
        sl = slice(i * J, (i + 1) * J)
        in_maps.append({
            "w": np.ascontiguousarray(W[:, sl]),
            "tbc": tbc,
            "fbc": fbc,
            "fpc": fpc,
            "tpc": tpc,
            "tnpc": tnpc,
            "hwpc": hwpc,
            "afbc": np.ascontiguousarray(
                np.broadcast_to(afreqs[sl][None, :], (P, J))).astype(np.float32),
            "bbc": np.ascontiguousarray(
                np.broadcast_to(b[sl][None, :], (P, J))).astype(np.float32),
        })
    res = run_bass_kernel_spmd(nc, in_maps, core_ids=list(range(8)))
    return np.concatenate([res.results[i]["out"] for i in range(8)], axis=1)


# revision 17
# speedup vs baseline: 1.0267x; 1.0267x over previous
"""Trainium2 Bass kernel for the AdaptiveGaussKronrod VJP quadrature problem.

Math (reference, flattened over N = S*15 = 1920 quadrature nodes):
    phi = sin(t (x) freqs)                  [N, D]
    Z   = phi @ W + b                       [N, D]
    G   = (h*wk)_n * cos(t (x) afreqs) * (1 - tanh(Z)^2)
    out = phi^T @ G                         [D, D]

Sharding: output-column parallel over 8 cores (J = D/8 = 512 columns each).
Core i needs W[:, cols], b[cols], afreqs[cols], full freqs. No collectives:
each core's [D, 512] output block is independent; host concatenates.

Per-core pipeline (Tile framework, bf16 matmuls / fp32 accumulation):
  pass 1 (GEMM1): phi_T tiles ([d, n] layout) generated by ScalarE Sin
    activation in 640-wide n-blocks; Z accumulated in PSUM per n-row-tile;
    epilogue computes G tiles [n, 512] via Tanh / Sin(pi/2 - x) / DVE math.
  pass 2 (GEMM2): phi_N tiles ([n, d] layout) regenerated by ScalarE in
    1024-wide d-column blocks; out accumulated in PSUM; DMA to DRAM.
All constant broadcast/column tiles are pre-arranged on the host so device
DMAs are contiguous. ScalarE emission interleaves phi generation with the
per-block epilogues so the in-order engine never blocks the PE.
"""

import math

import numpy as np

D = 4096
S = 128
J = D // 8          # output columns per core
N = S * 15          # 1920 quadrature nodes
P = 128
KT = D // P         # 32 k-tiles over D
MT = N // P         # 15 m-tiles over N
OT = D // P         # 32 output row tiles

PT_BLK_M = 5                     # pass-1 n-blocks: 3 x 640 (5 m-tiles each)
PT_BLK_W = PT_BLK_M * P          # 640
PT_NBLK = MT // PT_BLK_M         # 3
PN_BLK_O = 4                     # pass-2 d-col blocks: 8 x 512 (4 o-tiles)
PN_BLK_W = PN_BLK_O * P          # 512
PN_NBLK = OT // PN_BLK_O         # 8

_NODES_NEG = np.array([-0.9914553711208126, -0.9491079123427585, -0.8648644233597691,
                       -0.7415311855993945, -0.5860872354676911, -0.4058451513773972,
                       -0.20778495500789848, 0.0])
_WK_HALF = np.array([0.022935322010529224, 0.06309209262997856, 0.10479001032225019,
                     0.14065325971552592, 0.1690047266392679, 0.19035057806478542,
                     0.20443294007529889, 0.20948214108472782])
GK_NODES = np.concatenate([-_NODES_NEG[:-1][::-1], _NODES_NEG])  # [15]
GK_WK = np.concatenate([_WK_HALF[:-1][::-1], _WK_HALF])          # [15]


def _host_constants():
    edges = np.linspace(0.0, 1.0, S + 1, dtype=np.float32)
    a_s, b_s = edges[:-1], edges[1:]
    h = (b_s - a_s) / 2.0
    c = (a_s + b_s) / 2.0
    t = (c[:, None] + h[:, None] * GK_NODES[None, :].astype(np.float32)).reshape(-1)
    hw = (h[:, None] * GK_WK[None, :].astype(np.float32)).reshape(-1)
    return t.astype(np.float32), hw.astype(np.float32)


def _patch_act_tables():
    """Force Sin AND Tanh to resolve to one table set (silu_and_others) so
    the act-table-load pass emits a single load instead of thrashing
    between trig_and_small and exp_and_others on every Sin<->Tanh switch."""
    import concourse.bacc as bacc_mod
    from concourse import mybir

    if getattr(bacc_mod, "_act_tables_pinned", False):
        return
    orig = bacc_mod.get_activation_tables
    Sin = mybir.ActivationFunctionType.Sin
    Tanh = mybir.ActivationFunctionType.Tanh

    def patched(arch):
        tabs = orig(arch)
        out = {}
        for name, funcs in tabs.items():
            if (Sin in funcs) and (Tanh in funcs):
                out[name] = funcs
            else:
                out[name] = funcs - {Sin, Tanh}
        return out

    bacc_mod.get_activation_tables = patched
    bacc_mod._act_tables_pinned = True


def build_bass():
    """Build and compile the per-core Bass graph (identical on all 8 cores)."""
    from contextlib import ExitStack

    import concourse.bass as bass
    import concourse.tile as tile
    from concourse import bacc, mybir

    _patch_act_tables()

    f32 = mybir.dt.float32
    bf16 = mybir.dt.bfloat16
    Sin = mybir.ActivationFunctionType.Sin
    Tanh = mybir.ActivationFunctionType.Tanh

    nc = bacc.Bacc("TRN2", target_bir_lowering=False, debug=False,
                   enable_asserts=False)

    w_ext = nc.dram_tensor("w", [D, J], f32, kind="ExternalInput")
    tbc_ext = nc.dram_tensor("tbc", [P, N], f32, kind="ExternalInput")
    fbc_ext = nc.dram_tensor("fbc", [P, D], bf16, kind="ExternalInput")
    fpc_ext = nc.dram_tensor("fpc", [P, KT], f32, kind="ExternalInput")
    tpc_ext = nc.dram_tensor("tpc", [P, MT], f32, kind="ExternalInput")
    tnpc_ext = nc.dram_tensor("tnpc", [P, MT], f32, kind="ExternalInput")
    hwpc_ext = nc.dram_tensor("hwpc", [P, MT], f32, kind="ExternalInput")
    afbc_ext = nc.dram_tensor("afbc", [P, J], f32, kind="ExternalInput")
    bbc_ext = nc.dram_tensor("bbc", [P, J], f32, kind="ExternalInput")
    out_ext = nc.dram_tensor("out", [D, J], f32, kind="ExternalOutput")

    with tile.TileContext(nc) as tc, ExitStack() as ctx:
        consts = ctx.enter_context(tc.tile_pool(name="consts", bufs=1))
        stage = ctx.enter_context(tc.tile_pool(name="stage", bufs=3))
        wsp = ctx.enter_context(tc.tile_pool(name="ws", bufs=KT))
        phip = ctx.enter_context(tc.tile_pool(name="phi", bufs=64))
        work = ctx.enter_context(tc.tile_pool(name="work", bufs=2))
        gp = ctx.enter_context(tc.tile_pool(name="g", bufs=MT))
        zps = ctx.enter_context(
            tc.tile_pool(name="zpsum", bufs=5, space=bass.MemorySpace.PSUM))
        ops = ctx.enter_context(
            tc.tile_pool(name="opsum", bufs=3, space=bass.MemorySpace.PSUM))

        # ---- PE warm-up: dummy matmuls so HAM reaches K=8/8 before the
        # real GEMM starts (~3.4us of sustained PE activity required) ----
        dummy = consts.tile([P, J], bf16, tag="dummy")
        nc.vector.memset(dummy[:], 0.0)
        wps = ops.tile([P, J], f32, tag="opsum", name="warmps")
        for i in range(64):
            nc.tensor.matmul(wps[:, 0:64], lhsT=dummy[:, 0:128],
                             rhs=dummy[:, 128:192], start=True, stop=True)

        # ---- constants (host-prearranged, contiguous DMAs) ----
        t_bc = consts.tile([P, N], f32, tag="t_bc")
        nc.sync.dma_start(t_bc[:], tbc_ext[:])
        f_pc = consts.tile([P, KT], f32, tag="f_pc")
        nc.sync.dma_start(f_pc[:], fpc_ext[:])
        zero_c = consts.tile([P, 1], f32, tag="zero_c")
        nc.vector.memset(zero_c[:], 0.0)
        halfpi_c = consts.tile([P, 1], f32, tag="halfpi_c")
        nc.vector.memset(halfpi_c[:], math.pi / 2)
        # first ScalarE op: pulls the ACT table load to kernel start
        nc.scalar.activation(halfpi_c[:], zero_c[:], Sin, bias=zero_c[:])
        nc.vector.memset(halfpi_c[:], math.pi / 2)

        # ---- W shard: DMA f32, convert to bf16 (DVE) ----
        ws = []
        for k in range(KT):
            stg = stage.tile([P, J], f32, tag="stage512", name=f"wstg{k}")
            nc.sync.dma_start(stg[:], w_ext[k * P:(k + 1) * P, :])
            wb = wsp.tile([P, J], bf16, tag="ws", name=f"ws{k}")
            nc.vector.tensor_copy(wb[:], stg[:])
            ws.append(wb)

        # remaining constants (needed from the first epilogue onward)
        af_bc = consts.tile([P, J], f32, tag="af_bc")
        nc.sync.dma_start(af_bc[:], afbc_ext[:])
        b_bc = consts.tile([P, J], f32, tag="b_bc")
        nc.sync.dma_start(b_bc[:], bbc_ext[:])
        t_pc = consts.tile([P, MT], f32, tag="t_pc")
        nc.sync.dma_start(t_pc[:], tpc_ext[:])
        tn_pc = consts.tile([P, MT], f32, tag="tn_pc")
        nc.sync.dma_start(tn_pc[:], tnpc_ext[:])
        hw_pc = consts.tile([P, MT], f32, tag="hw_pc")
        nc.sync.dma_start(hw_pc[:], hwpc_ext[:])
        freqs_bc = consts.tile([P, D], bf16, tag="freqs_bc")
        nc.sync.dma_start(freqs_bc[:], fbc_ext[:])

        # ---- builders ----
        def gen_phit_block(blk):
            n0 = blk * PT_BLK_W
            tiles = []
            for k in range(KT):
                pt = phip.tile([P, PT_BLK_W], bf16, tag="phi",
                               name=f"pt{blk}_{k}")
                nc.scalar.activation(pt[:], t_bc[:, n0:n0 + PT_BLK_W], Sin,
                                     bias=zero_c[:], scale=f_pc[:, k:k + 1])
                tiles.append(pt)
            return tiles

        def gen_phin_block(blk):
            c0 = blk * PN_BLK_W
            tiles = []
            for n in range(MT):
                pn = phip.tile([P, PT_BLK_W], bf16, tag="phi",
                               name=f"pn{blk}_{n}")
                nc.scalar.activation(pn[:, :PN_BLK_W],
                                     freqs_bc[:, c0:c0 + PN_BLK_W], Sin,
                                     bias=zero_c[:], scale=t_pc[:, n:n + 1])
                tiles.append(pn)
            return tiles

        def mm_block(blk, phiT):
            zt = [zps.tile([P, J], f32, tag="zpsum", name=f"zt{blk}_{i}")
                  for i in range(PT_BLK_M)]
            for k in range(KT):
                for ml in range(PT_BLK_M):
                    nc.tensor.matmul(zt[ml][:],
                                     lhsT=phiT[k][:, ml * P:(ml + 1) * P],
                                     rhs=ws[k][:],
                                     start=(k == 0), stop=(k == KT - 1))
            return zt

        def epilogue(blk, zt):
            # z-adds first: frees all PSUM banks for the next block ASAP
            zs = []
            for ml in range(PT_BLK_M):
                z = work.tile([P, J], f32, tag="z", name=f"z{blk}_{ml}")
                nc.vector.tensor_add(z[:], zt[ml][:], b_bc[:])
                zs.append(z)
            for ml in range(PT_BLK_M):
                m = blk * PT_BLK_M + ml
                z = zs[ml]
                nc.scalar.activation(z[:], z[:], Tanh, bias=zero_c[:])
                c = work.tile([P, J], f32, tag="c", name=f"c{blk}_{ml}")
                nc.scalar.activation(c[:], af_bc[:], Sin,
                                     scale=tn_pc[:, m:m + 1], bias=halfpi_c[:])
                s = work.tile([P, J], f32, tag="s", name=f"s{blk}_{ml}")
                nc.vector.tensor_mul(s[:], z[:], z[:])
                nc.vector.tensor_scalar(s[:], s[:], -1.0, 1.0,
                                        mybir.AluOpType.mult, mybir.AluOpType.add)
                nc.vector.tensor_mul(c[:], c[:], s[:])
                g = gp.tile([P, J], bf16, tag="g", name=f"g{m}")
                nc.vector.tensor_scalar_mul(g[:], c[:], hw_pc[:, m:m + 1])
                g_tiles[m] = g

        g_tiles = [None] * MT
        # emission order chosen so the in-order ScalarE stream is:
        #   g0 g1 | e0 g2 | e1 p2g0 | e2 p2g1 | p2g2 p2g3
        phiT0 = gen_phit_block(0)
        phiT1 = gen_phit_block(1)
        zt0 = mm_block(0, phiT0)
        epilogue(0, zt0)
        phiT2 = gen_phit_block(2)
        zt1 = mm_block(1, phiT1)
        epilogue(1, zt1)
        phiN = {0: gen_phin_block(0)}
        zt2 = mm_block(2, phiT2)
        phiN[1] = gen_phin_block(1)
        epilogue(2, zt2)

        # ---- pass 2: GEMM2 (out = phi^T @ G) ----
        for blk in range(PN_NBLK):
            pn = phiN.pop(blk)
            if blk + 2 < PN_NBLK:
                phiN[blk + 2] = gen_phin_block(blk + 2)
            for ol in range(PN_BLK_O):
                o = blk * PN_BLK_O + ol
                op = ops.tile([P, J], f32, tag="opsum", name=f"op{o}")
                for n in range(MT):
                    nc.tensor.matmul(op[:],
                                     lhsT=pn[n][:, ol * P:(ol + 1) * P],
                                     rhs=g_tiles[n][:],
                                     start=(n == 0), stop=(n == MT - 1))
                ostg = stage.tile([P, J], f32, tag="stage512", name=f"ostg{o}")
                nc.vector.tensor_copy(ostg[:], op[:])
                nc.sync.dma_start(out_ext[o * P:(o + 1) * P, :], ostg[:])

    nc.compile()
    return nc


_CACHE = {}


def _get_nc():
    if "nc" not in _CACHE:
        _CACHE["nc"] = build_bass()
    return _CACHE["nc"]


def kernel(W, b, freqs, afreqs):
    import ml_dtypes
    from concourse.bass_utils import run_bass_kernel_spmd

    W = np.ascontiguousarray(np.asarray(W, dtype=np.float32))
    b = np.asarray(b, dtype=np.float32)
    freqs = np.asarray(freqs, dtype=np.float32)
    afreqs = np.asarray(afreqs, dtype=np.float32)
    t, hw = _host_constants()

    tbc = np.ascontiguousarray(np.broadcast_to(t[None, :], (P, N))).astype(np.float32)
    tpc = np.ascontiguousarray(t.reshape(MT, P).T)
    tnpc = np.ascontiguousarray((-t).reshape(MT, P).T)
    hwpc = np.ascontiguousarray(hw.reshape(MT, P).T)
    fbc = np.ascontiguousarray(
        np.broadcast_to(freqs[None, :], (P, D))).astype(ml_dtypes.bfloat16)
    fpc = np.ascontiguousarray(freqs.reshape(KT, P).T)

    nc = _get_nc()
    in_maps = []
    for i in range(8):
        sl = slice(i * J, (i + 1) * J)
        in_maps.append({
            "w": np.ascontiguousarray(W[:, sl]),
            "tbc": tbc,
            "fbc": fbc,
            "fpc": fpc,
            "tpc": tpc,
            "tnpc": tnpc,
            "hwpc": hwpc,
            "afbc": np.ascontiguousarray(
                np.broadcast_to(afreqs[sl][None, :], (P, J))).astype(np.float32),
            "bbc": np.ascontiguousarray(
                np.broadcast_to(b[sl][None, :], (P, J))).astype(np.float32),
        })
    res = run_bass_kernel_spmd(nc, in_maps, core_ids=list(range(8)))
    return np.concatenate([res.results[i]["out"] for i in range(8)], axis=1)
